# revision 24
# baseline (speedup 1.0000x reference)
"""LlamaAttention (B=2, S=2048, H=4096, 32 q heads / 8 kv heads, RoPE, causal)
on 8 Trainium2 NeuronCores.

Sharding: data-parallel over batch (2) x tensor-parallel over heads (4).
Core c = b*4 + t handles batch b with q heads 8t..8t+7 and kv heads 2t..2t+1.
Each core computes a partial output y_c = attn_out_local @ wo_local^T
([S, H], bf16); the host upcasts and sums the 4 TP partials per batch.

All matmuls bf16 inputs / fp32 PSUM accumulation. All weight/activation
DRAM tensors are pre-interleaved on the host so each DMA is a direct image
of its SBUF destination.

v4 structure (two fused phases, single hs pass, no DRAM spill):
  KVQ phase : per 512-token block tb, one pass over hs quarters computes
              K (2 kv heads) + V (vAug with ones column for the softmax
              denominator) + Q (8 heads in four 2-head PSUM quarter-passes)
              with fused RoPE. qT for ALL blocks stays resident in SBUF.
  ATTN+O    : per tb, causal attention (scores transposed, exp in PAIRED
              [128,1024] ACT tiles, multiplicative 0/1 diagonal masks, AV
              with vAug, normalize, PE transpose -> oTb) interleaved with
              the O-projection matmuls of block tb-1 (oTb consumed directly
              from SBUF; y rows stored bf16). The O matmuls give TensorE
              independent work while ScalarE produces exp tiles.
"""
import sys

sys.path.insert(0, "/opt/trn_rl_repo")

import numpy as np
import ml_dtypes

BF16 = ml_dtypes.bfloat16

B, S, H = 2, 2048, 4096
NH, NKV, HD = 32, 8, 128
THETA = 10000.0
SCALE = 1.0 / float(np.sqrt(HD))

N_CORES = 8
TP = 4
NH_L = NH // TP        # 8 local q heads
NKV_L = NKV // TP      # 2 local kv heads
GRP_L = NH_L // NKV_L  # 4 q heads per local kv head
TOKB = 512
NKC = H // 128         # 32 contraction chunks
NTB = S // TOKB        # 4 token blocks
NQC = S // 128         # 16 token chunks
VSTRIDE = 132          # per-chunk stride in vAug (129 used, pad for alignment)
HQ = NKC // 8          # 4 kc chunks per hs eighth tile

_NC_CACHE = {}


def _rope(nc, rp, psum, cos_sb, sinn_sb, tsl, outT, col0, f32):
    """RoPE on a [128(d), TOKB] fp32 PSUM block; writes bf16 to outT[:, col0:+TOKB].

    out[0:64]   = p[0:64]*cos - p[64:128]*sin
    out[64:128] = p[64:128]*cos + p[0:64]*sin
    (cos rows duplicated; sinn rows 0:64 pre-negated on host; fp16 tables.)
    """
    tcos = rp.tile([128, TOKB], f32, tag="tcos")
    nc.vector.tensor_mul(tcos[:], psum[:], cos_sb[:, tsl])
    trs = rp.tile([128, TOKB], f32, tag="trs")
    nc.vector.tensor_mul(trs[0:64, :], psum[64:128, :], sinn_sb[0:64, tsl])
    nc.vector.tensor_mul(trs[64:128, :], psum[0:64, :], sinn_sb[64:128, tsl])
    nc.vector.tensor_add(outT[:, col0: col0 + TOKB], tcos[:], trs[:])


def _build(reps=1, phases="full"):
    import concourse.mybir as mybir
    import concourse.tile as tile
    from concourse import bacc
    from contextlib import ExitStack

    dt = mybir.dt
    f32, bf16, f16 = dt.float32, dt.bfloat16, dt.float16
    af = mybir.ActivationFunctionType

    nc = bacc.Bacc("TRN2", target_bir_lowering=False, debug=False,
                   enable_asserts=True, num_devices=N_CORES)
    # all pre-interleaved on host: DMA source rows == SBUF partition images
    hs_d = nc.dram_tensor("hs", [NTB * 128, NKC * TOKB], bf16, kind="ExternalInput").ap()
    wq_d = nc.dram_tensor("wq", [128, NKC * NH_L * 128], bf16, kind="ExternalInput").ap()
    wk_d = nc.dram_tensor("wk", [128, NKC * NKV_L * 128], bf16, kind="ExternalInput").ap()
    wv_d = nc.dram_tensor("wv", [128, NKC * NKV_L * 128], bf16, kind="ExternalInput").ap()
    wo_d = nc.dram_tensor("wo", [128, NH_L * H], bf16, kind="ExternalInput").ap()
    cos_d = nc.dram_tensor("cosT", [128, S], f16, kind="ExternalInput").ap()
    sin_d = nc.dram_tensor("sinN", [128, S], f16, kind="ExternalInput").ap()
    msk_d = nc.dram_tensor("maskB", [128, 4 * TOKB], bf16, kind="ExternalInput").ap()
    id_d = nc.dram_tensor("ident", [128, 128], bf16, kind="ExternalInput").ap()
    y_d = nc.dram_tensor("y", [S, H], bf16, kind="ExternalOutput").ap()

    def emit(ctx, tc):
        persist = ctx.enter_context(tc.tile_pool(name="persist", bufs=1))
        kT = persist.tile([128, NKV_L * S], bf16, tag="kT")
        vA = persist.tile([128, NKV_L * NQC * VSTRIDE], bf16, tag="vA")
        nc.gpsimd.memset(vA[:], 1.0)
        qT = persist.tile([128, NH_L * S], bf16, tag="qT")

        emit_kvq(tc, kT, vA, qT)
        emit_attn_o(tc, kT, vA, qT)

    def emit_kvq(tc, kT, vA, qT):
        with tc.tile_pool(name="wkv", bufs=1) as wkvp, \
             tc.tile_pool(name="wq", bufs=1) as wqp, \
             tc.tile_pool(name="cs", bufs=1) as csp, \
             tc.tile_pool(name="rope", bufs=2) as rp, \
             tc.tile_pool(name="hs", bufs=10) as hsp, \
             tc.tile_pool(name="pk", bufs=2, space="PSUM") as pkp, \
             tc.tile_pool(name="pv", bufs=4, space="PSUM") as pvp, \
             tc.tile_pool(name="pq", bufs=2, space="PSUM") as pqp:
            wk_sb = wkvp.tile([128, NKC * NKV_L * 128], bf16, tag="wk")
            wv_sb = wkvp.tile([128, NKC * NKV_L * 128], bf16, tag="wv")
            # per-quarter chunks pipelined with the tb0 hs tiles: the first
            # K/V matmuls start after ~2MB instead of ~5.2MB of DMA
            wq_chunk = NKC * NKV_L * 128 // 4
            cos_sb = csp.tile([128, S], f16, tag="cos")
            sinn_sb = csp.tile([128, S], f16, tag="sinn")
            wq_sb = wqp.tile([128, NKC * NH_L * 128], bf16, tag="wq")

            for tb in range(NTB):
                tsl = slice(tb * TOKB, (tb + 1) * TOKB)
                hts = []
                for hq in range(8):
                    if tb == 0 and hq % 2 == 0:
                        csl = slice(hq // 2 * wq_chunk, (hq // 2 + 1) * wq_chunk)
                        nc.sync.dma_start(wk_sb[:, csl], wk_d[:, csl])
                        nc.sync.dma_start(wv_sb[:, csl], wv_d[:, csl])
                    ht = hsp.tile([128, HQ * TOKB], bf16, tag="hs",
                                  name=f"hs{tb}_{hq}")
                    nc.sync.dma_start(
                        ht[:], hs_d[tb * 128:(tb + 1) * 128,
                                    hq * HQ * TOKB:(hq + 1) * HQ * TOKB])
                    hts.append(ht)
                    if tb == 0 and hq == 7:
                        # late loads: Sync FIFO delivers wk/wv/hs first so
                        # TensorE starts early; cos/sin before the first
                        # k-RoPE, then wq in head-pair chunks arriving
                        # just-in-time for each Q quarter-pass
                        nc.sync.dma_start(cos_sb[:], cos_d[:])
                        nc.sync.dma_start(sinn_sb[:], sin_d[:])
                        hp = 2 * NKC * 128
                        for qp in range(4):
                            qsl = slice(qp * hp, (qp + 1) * hp)
                            nc.sync.dma_start(wq_sb[:, qsl], wq_d[:, qsl])

                pks = [pkp.tile([128, TOKB], f32, tag="pk", name=f"pk{tb}_{i}")
                       for i in range(NKV_L)]
                pvs = [pvp.tile([128, 256], f32, tag="pv", name=f"pv{tb}_{i}")
                       for i in range(4)]
                for kc in range(NKC):
                    ht = hts[kc // HQ]
                    hsl = slice((kc % HQ) * TOKB, (kc % HQ + 1) * TOKB)
                    for g in range(NKV_L):
                        c0 = kc * 256 + g * 128
                        nc.tensor.matmul(
                            pks[g][:], wk_sb[:, c0:c0 + 128], ht[:, hsl],
                            start=(kc == 0), stop=(kc == NKC - 1))
                    for s in range(4):
                        nc.tensor.matmul(
                            pvs[s][:],
                            ht[:, (kc % HQ) * TOKB + s * 128:
                                  (kc % HQ) * TOKB + (s + 1) * 128],
                            wv_sb[:, kc * 256:(kc + 1) * 256],
                            start=(kc == 0), stop=(kc == NKC - 1))
                for s in range(4):
                    qc = tb * 4 + s
                    for g in range(NKV_L):
                        c0 = (g * NQC + qc) * VSTRIDE
                        nc.vector.tensor_copy(
                            vA[:, c0:c0 + 128],
                            pvs[s][:, g * 128:(g + 1) * 128])
                for g in range(NKV_L):
                    _rope(nc, rp, pks[g], cos_sb, sinn_sb, tsl, kT,
                          g * S + tb * TOKB, f32)
                # Q: four 2-head quarter-passes over the resident hs tiles
                for qp in range(4):
                    pqs = [pqp.tile([128, TOKB], f32, tag="pq",
                                    name=f"pq{tb}_{qp}_{i}") for i in range(2)]
                    for kc in range(NKC):
                        ht = hts[kc // HQ]
                        hsl = slice((kc % HQ) * TOKB, (kc % HQ + 1) * TOKB)
                        for i in range(2):
                            h = qp * 2 + i
                            c0 = h * NKC * 128 + kc * 128
                            nc.tensor.matmul(
                                pqs[i][:], wq_sb[:, c0:c0 + 128], ht[:, hsl],
                                start=(kc == 0), stop=(kc == NKC - 1))
                    for i in range(2):
                        h = qp * 2 + i
                        _rope(nc, rp, pqs[i], cos_sb, sinn_sb, tsl, qT,
                              h * S + tb * TOKB, f32)

    def emit_attn_o(tc, kT, vA, qT):
        with tc.tile_pool(name="p2p", bufs=1) as p2p, \
             tc.tile_pool(name="exp", bufs=18) as ep, \
             tc.tile_pool(name="otb", bufs=2) as op, \
             tc.tile_pool(name="yr", bufs=2) as yrp, \
             tc.tile_pool(name="on", bufs=6) as onp, \
             tc.tile_pool(name="ps2", bufs=2, space="PSUM") as ps2, \
             tc.tile_pool(name="pso", bufs=2, space="PSUM") as pso, \
             tc.tile_pool(name="py", bufs=2, space="PSUM") as pyp:
            mask_sb = p2p.tile([128, 4 * TOKB], bf16, tag="mask")
            nc.sync.dma_start(mask_sb[:], msk_d[:])
            id_sb = p2p.tile([128, 128], bf16, tag="ident")
            nc.sync.dma_start(id_sb[:], id_d[:])
            wo_sb = p2p.tile([128, NH_L * H], bf16, tag="wo")
            nc.sync.dma_start(wo_sb[:], wo_d[:])

            oTbs = {}

            def attn_stages(tb):
                """Attention for q-block tb as a list of emission closures.

                Per head: stage1 emits scores + paired exp (+masks), stage2
                emits AV/normalize/transpose. O-proj units of tb-1 are
                interleaved between stages so TensorE has independent work
                while ScalarE produces the exp tiles.
                """
                oTb = op.tile([128, NH_L * TOKB], bf16, tag="oTb",
                              name=f"oTb{tb}")
                oTbs[tb] = oTb
                nkc = 4 * tb + 4
                state = {}

                def stage1(h):
                    g = h // GRP_L
                    exps = []  # per kc: (tile, col0)
                    for kc2 in range(nkc // 2):
                        pp = ps2.tile([128, 2 * TOKB], f32, tag="ps2")
                        for j in range(2):
                            kc = 2 * kc2 + j
                            nc.tensor.matmul(
                                pp[:, j * TOKB:(j + 1) * TOKB],
                                kT[:, g * S + kc * 128: g * S + (kc + 1) * 128],
                                qT[:, h * S + tb * TOKB: h * S + (tb + 1) * TOKB],
                                start=True, stop=True)
                        e = ep.tile([128, 2 * TOKB], bf16, tag="exp")
                        nc.scalar.activation(e[:], pp[:], af.Exp, scale=SCALE)
                        for j in range(2):
                            kc = 2 * kc2 + j
                            r = kc - 4 * tb
                            if r >= 0:  # diagonal band: 0/1 mask
                                nc.vector.tensor_mul(
                                    e[:, j * TOKB:(j + 1) * TOKB],
                                    e[:, j * TOKB:(j + 1) * TOKB],
                                    mask_sb[:, r * TOKB:(r + 1) * TOKB])
                            exps.append((e, j * TOKB))
                    state[h] = exps

                def stage2(h):
                    g = h // GRP_L
                    exps = state.pop(h)
                    for s2 in range(4):
                        qc = 4 * tb + s2
                        po = pso.tile([128, VSTRIDE], f32, tag="pso")
                        for kc in range(qc + 1):
                            et, col0 = exps[kc]
                            c0 = (g * NQC + kc) * VSTRIDE
                            nc.tensor.matmul(
                                po[:, 0:129],
                                et[:, col0 + s2 * 128: col0 + (s2 + 1) * 128],
                                vA[:, c0:c0 + 129],
                                start=(kc == 0), stop=(kc == qc))
                        rcp = onp.tile([128, 1], f32, tag="rcp")
                        nc.vector.reciprocal(rcp[:], po[:, 128:129])
                        on = onp.tile([128, 128], bf16, tag="on")
                        nc.vector.tensor_scalar_mul(on[:], po[:, 0:128], rcp[:])
                        pt = pso.tile([128, VSTRIDE], bf16, tag="pso",
                                      name="pt")
                        nc.tensor.transpose(pt[:, 0:128], on[:], id_sb[:])
                        nc.vector.tensor_copy(
                            oTb[:, h * TOKB + s2 * 128: h * TOKB + (s2 + 1) * 128],
                            pt[:, 0:128])

                stages = [lambda: stage1(0)]
                for h in range(1, NH_L):
                    stages.append(lambda h=h: stage1(h))
                    stages.append(lambda h=h - 1: stage2(h))
                stages.append(lambda: stage2(NH_L - 1))
                return stages

            def o_units(tb):
                """O-projection of block tb from its SBUF-resident oTb:
                32 units, each 8 accumulating matmuls -> 512 y columns."""
                oTb = oTbs[tb]
                units = []
                yrs = {}

                def unit(t_local, hb):
                    t = tb * 4 + t_local
                    if hb == 0:
                        yrs[t_local] = yrp.tile([128, H], bf16, tag="yr",
                                                name=f"yr{t}")
                    yr = yrs[t_local]
                    pyt = pyp.tile([128, 512], f32, tag="py")
                    for dc in range(NH_L):
                        nc.tensor.matmul(
                            pyt[:],
                            oTb[:, dc * TOKB + t_local * 128:
                                   dc * TOKB + (t_local + 1) * 128],
                            wo_sb[:, dc * H + hb * 512: dc * H + (hb + 1) * 512],
                            start=(dc == 0), stop=(dc == NH_L - 1))
                    nc.vector.tensor_copy(yr[:, hb * 512:(hb + 1) * 512], pyt[:])
                    if hb == H // 512 - 1:
                        nc.sync.dma_start(y_d[t * 128:(t + 1) * 128, :], yr[:])

                for t_local in range(4):
                    for hb in range(H // 512):
                        units.append(lambda t=t_local, b=hb: unit(t, b))
                return units

            units = []
            for tb in range(NTB):
                stages = attn_stages(tb)
                while stages or units:
                    if stages:
                        stages.pop(0)()
                    for _ in range(2):
                        if units:
                            units.pop(0)()
                units = o_units(tb)
            for u in units:
                u()

    with tile.TileContext(nc) as tc:
        if reps == 1:
            with ExitStack() as ctx:
                emit(ctx, tc)
        else:
            with tc.For_i(0, reps, 1):
                with ExitStack() as ctx:
                    emit(ctx, tc)
    nc.compile()
    return nc


def get_nc(reps=1):
    if reps not in _NC_CACHE:
        _NC_CACHE[reps] = _build(reps)
    return _NC_CACHE[reps]


def make_in_maps(hidden_states, position_ids, wq, wk, wv, wo):
    hidden_states = np.asarray(hidden_states, dtype=np.float32)
    position_ids = np.asarray(position_ids)
    wq = np.asarray(wq, dtype=np.float32)
    wk = np.asarray(wk, dtype=np.float32)
    wv = np.asarray(wv, dtype=np.float32)
    wo = np.asarray(wo, dtype=np.float32)

    j = np.arange(64, dtype=np.float64)
    invf = 1.0 / (THETA ** (2.0 * j / HD))       # [64]
    kp = np.arange(128)[:, None]
    qf = np.arange(TOKB)[None, :]
    maskB = np.empty((128, 4 * TOKB), dtype=BF16)
    for r in range(4):
        maskB[:, r * TOKB:(r + 1) * TOKB] = (qf >= kp + 128 * r).astype(BF16)
    ident = np.eye(128, dtype=BF16)

    def interleave(wT, n):
        # [H_in, n] fp32 -> [128, (H_in/128)*n] bf16; row p holds all
        # contraction chunks for partition p (direct SBUF image)
        hin = wT.shape[0]
        return np.ascontiguousarray(
            wT.reshape(hin // 128, 128, n).transpose(1, 0, 2).reshape(128, -1)
        ).astype(BF16)

    in_maps = []
    for c in range(N_CORES):
        b, t = divmod(c, TP)
        pos = position_ids[b].astype(np.float64)     # [S]
        freqs = pos[:, None] * invf[None, :]         # [S, 64]
        cos64 = np.cos(freqs).astype(np.float16).T   # [64, S]
        sin64 = np.sin(freqs).astype(np.float16).T
        cosT = np.ascontiguousarray(np.concatenate([cos64, cos64], axis=0))
        sinN = np.ascontiguousarray(np.concatenate([-sin64, sin64], axis=0))

        hsT = hidden_states[b].T                     # [H, S] fp32
        # rows (tb*128 + p), cols (kc*512 + c)
        hs_i = np.ascontiguousarray(
            hsT.reshape(NKC, 128, NTB, TOKB).transpose(2, 1, 0, 3)
            .reshape(NTB * 128, NKC * TOKB)).astype(BF16)

        # wq head-major: row p, col (h*NKC*128 + kc*128 + d) = wqT[kc*128+p, h*128+d]
        wq_T = np.ascontiguousarray(wq[t * NH_L * HD:(t + 1) * NH_L * HD, :].T)
        wq_i = np.ascontiguousarray(
            wq_T.reshape(NKC, 128, NH_L, HD).transpose(1, 2, 0, 3)
            .reshape(128, -1)).astype(BF16)

        in_maps.append({
            "hs": hs_i,
            "wq": wq_i,
            "wk": interleave(
                np.ascontiguousarray(wk[t * NKV_L * HD:(t + 1) * NKV_L * HD, :].T),
                NKV_L * HD),
            "wv": interleave(
                np.ascontiguousarray(wv[t * NKV_L * HD:(t + 1) * NKV_L * HD, :].T),
                NKV_L * HD),
            "wo": interleave(
                np.ascontiguousarray(wo[:, t * NH_L * HD:(t + 1) * NH_L * HD].T), H),
            "cosT": cosT,
            "sinN": sinN,
            "maskB": maskB,
            "ident": ident,
        })
    return in_maps


def gather_out(results):
    """results: list of 8 dicts with 'y' [S, H] bf16 partials -> [B, S, H] fp32."""
    out = np.zeros((B, S, H), dtype=np.float32)
    for c in range(N_CORES):
        b = c // TP
        out[b] += np.asarray(results[c]["y"], dtype=np.float32)
    return out


def kernel(**inputs):
    from concourse.bass_utils import run_bass_kernel_spmd

    nc = get_nc(reps=1)
    in_maps = make_in_maps(**inputs)
    res = run_bass_kernel_spmd(nc, in_maps, core_ids=list(range(N_CORES)))
    return gather_out(res.results)
